# revision 17
# baseline (speedup 1.0000x reference)
"""Trainium2 Bass kernel for nn_EncodingInputLayer (embedding_lookup).

Math background
---------------
The reference computes, per batch b:
    v   = one_hot(x[:, :20], 10).reshape(B, 200) @ fc_w.T + fc_b      (B, 9)
    v_map  = broadcast_to(v,      (48, 48, B, 9)).reshape(B, 9, 48, 48)
    o_map  = broadcast_to(others, (48, 48, B, 23)).reshape(B, 23, 48, 48)
    out = all_w conv1x1( concat(oh_w conv1x1 v_map + oh_b,
                                ot_w conv1x1 o_map + ot_b) ) + all_b

The broadcast+raw-reshape *scrambles* batches: flattened, v_map is just
tile(v.flatten(), 48*48).  Working the indexing through (B*9 = 8*2304,
B*23 = 2048*23, 2304 = 48*48) shows batch b's output depends only on
b mod 8:

    out[b] = Map[b % 8],     Map[m] = A1 @ V8m + A2 @ Wm + const
    A1 = all_w[:, :9] @ oh_w, A2 = all_w[:, 9:] @ ot_w

Each core k receives x rolled by -256*k batches, computes its Map[k]
(a single 32 x 2304 tile -- the ONLY unique data among its 256 output
batches), and writes just that.  The host replicates each map over its
256 batches (b = k, k+8, ..., 2040) -- pure data movement of redundant
bytes the device has no reason to materialize.

Device pipeline:
  1. x -> SBUF contiguously (128 x 2752B descriptors); partition p
     holds batches 16p..16p+15.  One-hot feature cols replicated 6x
     into xbr[p, 128u + 20c + f] = x[16p+u, f]  (bf16, vector+ACT).
  2. 16 PE transposes -> psum block pxt[20c+f, 128u+p] = x[16p+u, f].
  3. is_equal with per-partition class vectors -> masks
     M_h[20c+f, col] = [x == class(h,c)]  (exact 0/1 bf16; pad rows
     compare against -1 so they are 0).
  4. 32 small matmuls with masks STATIONARY (lhsT=M_h 128-col slice,
     rhs=packed fc_w (128, 9) bf16) accumulate v batch-major in psum:
     pvv[p, 16u + e] = v[16p + u, e]  (fc_b folded into map rhs).
  5. vv -> DRAM (true flat v order) -> 8 circular window rows of the
     f32r map rhs (the 9th window duplicates row 0 and is folded into
     the host lhsT).  W windows (23 rows) DMA directly from a
     DRAM-bounced others.flatten(); a tiled-fc_b row and a ones row
     come from host (fc_b enters the output as c1[o]*fc_b[s%9] since
     2304 = 0 mod 9 makes all v-windows share e = s mod 9).
  6. Fused f32r map matmul (contract 33, full rate at >=256 cols) x 5
     psum banks -> Map (32, 2304) f32; per-chunk copies to SBUF
     alternate vector/ACT; output DMA split in two 147KB halves.

The tiny weight foldings (A1, A2, const rows, fc_w re-pack -- a few
KB) are precomputed on host, like the per-core x roll.
"""

import numpy as np
from contextlib import ExitStack

import ml_dtypes

import concourse.bass as bass
import concourse.mybir as mybir
import concourse.tile as tile
from concourse import bacc
from concourse.bass_utils import run_bass_kernel_spmd
from concourse.masks import make_identity

F32 = mybir.dt.float32
F32R = mybir.dt.float32r
BF16 = mybir.dt.bfloat16

B = 2048
NF = 43           # flat features per batch
N1 = 20           # one-hot index features
NO = 23           # passthrough features
NCLS = 10         # classes per one-hot
EMB = 9
OUTC = 32
H = W = 48
S = H * W         # 2304
NCORES = 8
BPC = B // NCORES  # 256 output batches per core
OLEN = B * NO      # 47104
PB = B // 128      # 16 batches per partition in the contiguous x layout

# rhs row layout for the fused map matmul
RW = 0            # rows 0..22  : 23 circular o_flat windows (direct DMA)
RV = 23           # rows 23..30 : 8 circular v_flat windows (9th == row 0,
                  #               folded into the host lhsT)
RFCB = 31         # row 31      : fc_b[s%9]  (coeff = A1 row-sums)
RONE = 32         # row 32      : ones       (coeff = folded bias)
NR = 33


def _emit(nc: bass.Bass):
    x = nc.dram_tensor("x", [B, NF], F32, kind="ExternalInput").ap()
    # packed per-partition params: [cls0 f32][cls1 f32][fcw0 9xbf16]
    # [fcw1 9xbf16][pad to 48B][lhsT row 64B] -> (128, 28) f32
    pf_d = nc.dram_tensor("pf", [128, 28], F32, kind="ExternalInput").ap()
    extra = nc.dram_tensor("extra", [2, S], BF16, kind="ExternalInput").ap()
    out = nc.dram_tensor("out", [OUTC, S], F32, kind="ExternalOutput").ap()
    o_flat = nc.dram_tensor("o_flat", [OLEN], BF16).ap()

    with ExitStack() as ctx:
        tc = ctx.enter_context(tile.TileContext(nc))
        consts = ctx.enter_context(tc.tile_pool(name="consts", bufs=1))
        ps_xt = ctx.enter_context(tc.tile_pool(name="ps_xt", bufs=1, space="PSUM"))
        ps_vv = ctx.enter_context(tc.tile_pool(name="ps_vv", bufs=1, space="PSUM"))
        ps_map = ctx.enter_context(tc.tile_pool(name="ps_map", bufs=1, space="PSUM"))

        # --- input DMAs ---------------------------------------------------
        # x first, bounce second on the same queue: x's 128 descriptors
        # lead the rings, the bounce's 2048 small ones follow.
        xc = consts.tile([128, PB * NF], F32)
        nc.sync.dma_start(xc[:, :], x.rearrange("(p u) f -> p (u f)", u=PB))

        # params (scalar queue, issue immediately)
        pf = consts.tile([128, 28], F32)
        nc.gpsimd.dma_start(pf[:, :], pf_d)
        cls0, cls1 = pf[:, 0:1], pf[:, 1:2]
        pfb = pf.bitcast(BF16)
        fcw0, fcw1 = pfb[:, 4:13], pfb[:, 13:22]
        lt = pfb[0:NR, 24:24 + OUTC]

        # map rhs (33, 2304) bf16; W windows + host rows land via DMA
        rhs = consts.tile([NR, S], BF16)
        nc.gpsimd.dma_start(rhs[RFCB:RFCB + 2, :], extra)

        identb = consts.tile([128, 128], BF16)
        nc.gpsimd.memset(identb[:, :], 0.0)  # reserve gpsimd early slot
        make_identity(nc, identb)

        # --- one-hot feature columns, replicated 6x, bf16 ----------------
        # xbr[p, 128u + 20c + f] = x[16p + u, f]   (cols 120..127 pad)
        xbr = consts.tile([128, PB * 128], BF16)
        xbr_v = xbr.rearrange("p (u k) -> p u k", k=128)
        xc_v = xc.rearrange("p (u f) -> p u f", f=NF)
        nc.gpsimd.memset(xbr_v[:, :, 6 * N1:128], 0.0)
        for r in range(4):
            nc.vector.tensor_copy(xbr_v[:, :, N1 * r:N1 * (r + 1)],
                                  xc_v[:, :, 0:N1])
        for r in range(4, 6):
            nc.scalar.copy(xbr_v[:, :, N1 * r:N1 * (r + 1)],
                           xc_v[:, :, 0:N1])

        # others -> bf16 in SBUF, then bounce to DRAM in true batch order:
        # o_flat[368p + 23u + n] = x[16p + u, 20 + n]  (contiguous/partition)
        xcob = consts.tile([128, PB * NO], BF16)
        nc.vector.tensor_copy(xcob.rearrange("p (u n) -> p u n", n=NO),
                              xc_v[:, :, N1:NF])
        nc.sync.dma_start(o_flat.rearrange("(p c) -> p c", c=PB * NO),
                          xcob[:, :])

        # W window rows: row j = o_flat[(2304j) % 47104 ...]
        nc.scalar.dma_start(rhs[0:20, :],
                            o_flat[0:20 * S].rearrange("(j s) -> j s", s=S))
        nc.scalar.dma_start(rhs[20:21, 0:OLEN - 20 * S],
                            o_flat[20 * S:OLEN][None, :])
        nc.scalar.dma_start(rhs[20:21, OLEN - 20 * S:S],
                            o_flat[0:S - (OLEN - 20 * S)][None, :])
        nc.scalar.dma_start(rhs[21:23, :],
                            o_flat[1280:1280 + 2 * S].rearrange("(j s) -> j s", s=S))

        # --- 16 PE transposes: replicated feature-major block in psum ----
        # pxt[20c + f, 128u + p] = x[16p + u, f]   (bf16, 2 banks)
        pxt_a = ps_xt.tile([128, B // 2], BF16, tag="xta", name="pxt_a")
        pxt_b = ps_xt.tile([128, B // 2], BF16, tag="xtb", name="pxt_b")
        pxt_h = [pxt_a, pxt_b]
        for u in range(PB):
            dst = pxt_h[u // 8]
            nc.tensor.transpose(dst[:, 128 * (u % 8):128 * (u % 8 + 1)],
                                xbr[:, 128 * u:128 * (u + 1)], identb[:, :])

        # --- class-packed one-hot masks (bf16, exact 0/1) ----------------
        # M_h[20c + f, col] = [x == cls_h[20c + f]]; pad rows vs -1 -> 0
        m0 = consts.tile([128, B], BF16)
        m1 = consts.tile([128, B], BF16)
        for h, (lo, hi) in enumerate(((0, B // 2), (B // 2, B))):
            nc.vector.tensor_scalar(out=m0[:, lo:hi], in0=pxt_h[h][:, :],
                                    scalar1=cls0, scalar2=None,
                                    op0=mybir.AluOpType.is_equal)
            nc.vector.tensor_scalar(out=m1[:, lo:hi], in0=pxt_h[h][:, :],
                                    scalar1=cls1, scalar2=None,
                                    op0=mybir.AluOpType.is_equal)

        # --- v in batch-major psum via mask-stationary matmuls -----------
        # pvv[p, 16u + e] = v[16p + u, e]  (no fc_b); all m0 halves first
        # so the second mask's is_equal never blocks the PE queue.
        pvv = ps_vv.tile([128, PB * 16], F32, tag="vv")
        for u in range(PB):
            nc.tensor.matmul(pvv[:, 16 * u:16 * u + EMB],
                             lhsT=m0[:, 128 * u:128 * (u + 1)],
                             rhs=fcw0, start=True, stop=False)
            nc.tensor.matmul(pvv[:, 16 * u:16 * u + EMB],
                             lhsT=m1[:, 128 * u:128 * (u + 1)],
                             rhs=fcw1, start=False, stop=True)

        # vv[p, 9u + e] = v[16p + u, e]; per-partition flat = v_flat chunk
        vv = consts.tile([128, PB * EMB], BF16)
        nc.vector.tensor_copy(
            vv.rearrange("p (u e) -> p u e", e=EMB),
            pvv.rearrange("p (u e) -> p u e", e=16)[:, :, 0:EMB],
        )

        # v windows: vv's linear element order (partition-major) equals
        # the linear order of the 8 window rows, so one DMA re-chunks
        # 128x144 -> 8x2304 (both APs canonical partition-leading).
        nc.sync.dma_start(rhs[RV:RV + 8, :], vv[:, :], single_packet=True)

        # --- fused f32r map matmul + output -------------------------------
        msb = consts.tile([OUTC, S], F32)
        pmaps = [ps_map.tile([OUTC, 512], F32, tag=f"map{ch}",
                             name=f"pmap{ch}")
                 for ch in range(5)]
        for ch in range(5):
            sz = 512 if ch < 4 else S - 4 * 512
            sl = slice(512 * ch, 512 * ch + sz)
            nc.tensor.matmul(pmaps[ch][:, 0:sz], lhsT=lt, rhs=rhs[:, sl],
                             start=True, stop=True)
        for ch in range(5):
            sz = 512 if ch < 4 else S - 4 * 512
            sl = slice(512 * ch, 512 * ch + sz)
            if ch % 2 == 0:
                nc.vector.tensor_copy(msb[:, sl], pmaps[ch][:, 0:sz])
            else:
                nc.scalar.copy(msb[:, sl], pmaps[ch][:, 0:sz])
        nc.sync.dma_start(out[:, 0:1024], msb[:, 0:1024])
        nc.gpsimd.dma_start(out[:, 1024:2048], msb[:, 1024:2048])
        nc.scalar.dma_start(out[:, 2048:S], msb[:, 2048:S])

    return nc


_NC_CACHE: dict = {}


def _get_nc():
    if "nc" not in _NC_CACHE:
        nc = bacc.Bacc("TRN2", target_bir_lowering=False, debug=False,
                       num_devices=NCORES)
        _emit(nc)
        nc.compile()
        _NC_CACHE["nc"] = nc
    return _NC_CACHE["nc"]


def _host_params(fc_w, fc_b, oh_w, oh_b, ot_w, ot_b, all_w, all_b):
    """Fold the tiny channel-mixing weights (host-side setup, a few KB)."""
    fc_w = np.asarray(fc_w, np.float32)
    fc_b = np.asarray(fc_b, np.float32)
    all_w = np.asarray(all_w, np.float32)
    A1 = all_w[:, :EMB] @ np.asarray(oh_w, np.float32)        # (32, 9)
    A2 = all_w[:, EMB:] @ np.asarray(ot_w, np.float32)        # (32, 23)
    tC = all_w @ np.concatenate([np.asarray(oh_b, np.float32),
                                 np.asarray(ot_b, np.float32)]) \
        + np.asarray(all_b, np.float32)                        # (32,)
    c1 = A1.sum(axis=1)                                        # (32,)
    A1T = A1.T.copy()
    A1T[0] += A1T[8]          # window 8 == window 0 (wrap): fold coeff
    lhsT = np.concatenate([A2.T, A1T[0:8], c1[None, :], tC[None, :]], axis=0)
    # fcwcf[h, 20c + f, e] = fc_w[e, 10f + c + 6h]; pad rows zero
    t = fc_w.reshape(EMB, N1, NCLS).transpose(2, 1, 0)         # (10, 20, 9)
    fcw = np.zeros((2, 128, EMB), np.float32)
    fcw[0, :6 * N1] = t[0:6].reshape(6 * N1, EMB)
    fcw[1, :4 * N1] = t[6:10].reshape(4 * N1, EMB)
    fcwb = fcw.astype(ml_dtypes.bfloat16)
    clsv = np.full((2, 128), -1.0, np.float32)
    for h in range(2):
        for c in range(6 if h == 0 else 4):
            clsv[h, N1 * c:N1 * (c + 1)] = c + 6 * h
    # packed (128, 112B): [cls0 f32][cls1 f32][fcw0 18B][fcw1 18B]
    # [pad to 48B][lhsT row 64B]
    pf = np.zeros((128, 112), np.uint8)
    pf[:, 0:4] = clsv[0, :, None].view(np.uint8).reshape(128, 4)
    pf[:, 4:8] = clsv[1, :, None].view(np.uint8).reshape(128, 4)
    pf[:, 8:26] = fcwb[0].view(np.uint8).reshape(128, 18)
    pf[:, 26:44] = fcwb[1].view(np.uint8).reshape(128, 18)
    ltb = lhsT.astype(ml_dtypes.bfloat16)
    pf[:NR, 48:112] = ltb.view(np.uint8).reshape(NR, 64)
    extra = np.stack([np.tile(fc_b, S // EMB),
                      np.ones(S, np.float32)], axis=0)         # (2, 2304)
    return {
        "pf": np.ascontiguousarray(pf.view(np.float32)),
        "extra": np.ascontiguousarray(extra.astype(ml_dtypes.bfloat16)),
    }


def _build_in_maps(x, fc_w, fc_b, oh_w, oh_b, ot_w, ot_b, all_w, all_b):
    xf = np.ascontiguousarray(np.asarray(x, dtype=np.float32).reshape(B, NF))
    params = _host_params(fc_w, fc_b, oh_w, oh_b, ot_w, ot_b, all_w, all_b)
    return [
        {"x": np.ascontiguousarray(np.roll(xf, -BPC * k, axis=0)), **params}
        for k in range(NCORES)
    ]


def kernel(x, fc_w, fc_b, oh_w, oh_b, ot_w, ot_b, all_w, all_b):
    nc = _get_nc()
    in_maps = _build_in_maps(x, fc_w, fc_b, oh_w, oh_b, ot_w, ot_b,
                             all_w, all_b)
    res = run_bass_kernel_spmd(nc, in_maps, list(range(NCORES)))
    full = np.empty((B, OUTC, H, W), dtype=np.float32)
    for k in range(NCORES):
        mk = res.results[k]["out"].reshape(1, OUTC, H, W)
        full[k::NCORES] = mk  # broadcast: all 256 batches share Map[k]
    return full
